# revision 16
# baseline (speedup 1.0000x reference)
"""Binarized MLP forward (BinaryConnect, training-mode BatchNorm) on 8 TRN2 cores.

Strategy: data-parallel over the batch (16384 -> 8 x 2048), weights replicated.
All activations kept TRANSPOSED on device ([features, batch]) so that
 - matmuls use binarized weights as the stationary operand,
 - BatchNorm stats are free-axis reductions (bn_stats on VectorE),
 - BN apply + ReLU is a single per-partition scale/bias activation on ScalarE.
Per-feature batch statistics are all-reduced across the 8 cores (8 KB/layer),
split into an early group (features 0..895, overlapped with the layer tail)
and a late group (last 128 features) to keep the boundary short.
Matmuls run in float32r (full PE rate at N=512; binarized +-1 weights exact).
"""
import os
import numpy as np

import concourse.bass as bass
import concourse.bacc as bacc
import concourse.tile as tile
import concourse.mybir as mybir
from concourse.bass_utils import run_bass_kernel_spmd

N_CORES = 8
B_TOT = 16384
BPC = B_TOT // N_CORES  # 2048 batch rows per core
NB = BPC // 512  # 4 free-dim tiles of 512
D_IN, H, D_OUT = 784, 1024, 10
D_IN_PAD = 896  # pad 784 -> 7 full k-tiles of 128
KT1 = D_IN_PAD // 128
NJ = H // 128  # 8 feature tiles per hidden layer
BN_EPS = 1e-5

f32 = mybir.dt.float32
f32r = mybir.dt.float32r
i32 = mybir.dt.int32
AF = mybir.ActivationFunctionType
ALU = mybir.AluOpType

# t_vec scratch layout (free-dim float offsets)
V_S = 0       # BN scale per feature (8)
V_T = 8       # BN shift per feature (8)
V_M = 16      # mean
V_E2 = 24
V_VU = 32     # var, then var+eps
V_SQ = 40     # sqrt(var+eps)
V_R = 48      # rsqrt
V_TMP = 56
V_TMP2 = 64


def build(nc):
    xT = nc.dram_tensor("xT", [KT1, NB, 128, 512], f32r, kind="ExternalInput")
    w1t = nc.dram_tensor("w1t", [D_IN_PAD, H], f32, kind="ExternalInput")
    w2t = nc.dram_tensor("w2t", [H, H], f32, kind="ExternalInput")
    w3t = nc.dram_tensor("w3t", [H, H], f32, kind="ExternalInput")
    w4t = nc.dram_tensor("w4t", [H, D_OUT], f32, kind="ExternalInput")
    gbp = nc.dram_tensor("gbp", [128, 6, 8], f32, kind="ExternalInput")
    outT = nc.dram_tensor("outT", [D_OUT, BPC], f32, kind="ExternalOutput")

    rg = [list(range(N_CORES))]

    with tile.TileContext(nc) as tc:
        with (
            tc.tile_pool(name="hp", bufs=2) as hpool,
            tc.tile_pool(name="wp", bufs=2) as wpool,
            tc.tile_pool(name="w4p", bufs=1) as w4pool,
            tc.tile_pool(name="stg", bufs=4) as stgpool,
            tc.tile_pool(name="outp", bufs=2) as outpool,
            tc.tile_pool(name="msc", bufs=1) as mpool,
            tc.tile_pool(name="ps", bufs=8, space="PSUM") as pspool,
            tc.tile_pool(name="dram", bufs=1, space="DRAM") as dpool,
        ):
            t_stats = mpool.tile([128, 192], f32, name="t_stats")
            t_part = mpool.tile([128, 16], f32, name="t_part")
            t_gst = mpool.tile([128, 16], f32, name="t_gst")
            t_vec = mpool.tile([128, 72], f32, name="t_vec")
            t_gb = mpool.tile([128, 48], f32, name="t_gb")

            # --- warmup collective: absorb first-call ncfw/algorithm cost.
            # Pure DRAM->DRAM with unread output: zero coupling with the
            # compute DMA queues or SBUF dependency tracking.
            with nc.named_scope("warmup_ar"):
                win = dpool.tile([128, 2], f32, name="warm_in")
                wout = dpool.tile([128, 2], f32, name="warm_out", addr_space="Shared")
                nc.gpsimd.collective_compute(
                    "AllReduce", ALU.add, replica_groups=rg,
                    ins=[win[:].opt()], outs=[wout[:].opt()],
                )

            def prep_w(wt_dram, Wtile, nkt, tag_suffix):
                """DMA raw transposed weights into staging (half-tiles for a
                finer DMA/Sign pipeline), binarize (Sign)."""
                for k in range(nkt):
                    for hh in range(2):
                        stg = stgpool.tile(
                            [128, H // 2], f32, name=f"stg_{tag_suffix}_{k}_{hh}", tag="stg"
                        )
                        nc.sync.dma_start(
                            stg[:],
                            wt_dram[k * 128 : (k + 1) * 128, hh * 512 : (hh + 1) * 512],
                        )
                        half = Wtile[:, k, hh * 512 : (hh + 1) * 512]
                        nc.gpsimd.tensor_scalar(
                            stg[:], stg[:], 0.0, None, op0=ALU.is_ge
                        )
                        nc.gpsimd.tensor_scalar(
                            half, stg[:], 2.0, -1.0, op0=ALU.mult, op1=ALU.add
                        )

            # --- input loads, in first-consumer order: the layer-1 j0 column
            # needs W1s[k] and xT[b=0, k] for every k first.
            xT_t = hpool.tile([128, KT1, NB, 512], f32r, name="xT_t", tag="h")
            W1s = wpool.tile([128, KT1, H], f32r, name="W1s", tag="w")
            with nc.named_scope("w1prep"):
                for k in range(KT1):
                    for hh in range(2):
                        stg = stgpool.tile(
                            [128, H // 2], f32, name=f"stg_w1_{k}_{hh}", tag="stg"
                        )
                        nc.sync.dma_start(
                            stg[:],
                            w1t[k * 128 : (k + 1) * 128, hh * 512 : (hh + 1) * 512],
                        )
                        half = W1s[:, k, hh * 512 : (hh + 1) * 512]
                        nc.gpsimd.tensor_scalar(
                            stg[:], stg[:], 0.0, None, op0=ALU.is_ge
                        )
                        nc.gpsimd.tensor_scalar(
                            half, stg[:], 2.0, -1.0, op0=ALU.mult, op1=ALU.add
                        )
                    nc.sync.dma_start(xT_t[:, k, 0], xT[k, 0])
            with nc.named_scope("xload"):
                for b in range(1, NB):
                    for k in range(KT1):
                        nc.sync.dma_start(xT_t[:, k, b], xT[k, b])
            nc.sync.dma_start(t_gb[:], gbp[:].rearrange("p a b -> p (a b)"))

            W4s = w4pool.tile([128, NJ, D_OUT], f32r, name="W4s")
            with nc.named_scope("w4prep"):
                stg4 = stgpool.tile([128, NJ, D_OUT], f32, name="stg_w4", tag="stg")
                nc.sync.dma_start(
                    stg4[:], w4t[:].rearrange("(kt p) c -> p kt c", p=128)
                )
                sview = stg4[:].rearrange("p a b -> p (a b)")
                nc.gpsimd.tensor_scalar(sview, sview, 0.0, None, op0=ALU.is_ge)
                nc.gpsimd.tensor_scalar(
                    W4s[:].rearrange("p a b -> p (a b)"), sview, 2.0, -1.0,
                    op0=ALU.mult, op1=ALU.add,
                )

            h1 = hpool.tile([128, NJ, NB, 512], f32r, name="h1", tag="h")
            h2 = hpool.tile([128, NJ, NB, 512], f32r, name="h2", tag="h")
            h3 = hpool.tile([128, NJ, NB, 512], f32r, name="h3", tag="h")

            def collective_group(li, j_lo, j_hi, gtag):
                """All-reduce partial stats for feature tiles [j_lo, j_hi)."""
                n = (j_hi - j_lo) * 2
                with nc.named_scope(f"L{li}_ar{gtag}"):
                    cin = dpool.tile([128, n], f32, name=f"cin{li}{gtag}")
                    cout = dpool.tile(
                        [128, n], f32, name=f"cout{li}{gtag}", addr_space="Shared"
                    )
                    nc.gpsimd.dma_start(
                        cin[:], t_part[:, j_lo * 2 : j_hi * 2]
                    )
                    nc.gpsimd.collective_compute(
                        "AllReduce", ALU.add, replica_groups=rg,
                        ins=[cin[:].opt()], outs=[cout[:].opt()],
                    )
                    nc.gpsimd.dma_start(t_gst[:, j_lo * 2 : j_hi * 2], cout[:])

            def st_group(li, j_lo, j_hi, gtag):
                """s = g*rsqrt(v+eps), t = b - m*s for feature tiles [j_lo, j_hi)."""
                with nc.named_scope(f"L{li}_st{gtag}"):
                    gview = t_gst[:, j_lo * 2 : j_hi * 2].rearrange(
                        "p (j c) -> p j c", c=2
                    )
                    mm = t_vec[:, V_M + j_lo : V_M + j_hi]
                    e2 = t_vec[:, V_E2 + j_lo : V_E2 + j_hi]
                    vu = t_vec[:, V_VU + j_lo : V_VU + j_hi]
                    sq = t_vec[:, V_SQ + j_lo : V_SQ + j_hi]
                    rr = t_vec[:, V_R + j_lo : V_R + j_hi]
                    tp2 = t_vec[:, V_TMP2 + j_lo : V_TMP2 + j_hi]
                    sv = t_vec[:, V_S + j_lo : V_S + j_hi]
                    tv = t_vec[:, V_T + j_lo : V_T + j_hi]
                    g_sl = t_gb[:, (li - 1) * 16 + j_lo : (li - 1) * 16 + j_hi]
                    b_sl = t_gb[:, (li - 1) * 16 + 8 + j_lo : (li - 1) * 16 + 8 + j_hi]
                    nc.vector.tensor_scalar(mm, gview[:, :, 0], 1.0 / N_CORES, None, op0=ALU.mult)
                    nc.vector.tensor_scalar(e2, gview[:, :, 1], 1.0 / N_CORES, None, op0=ALU.mult)
                    nc.vector.tensor_tensor(tp2, mm, mm, op=ALU.mult)
                    nc.vector.tensor_tensor(vu, e2, tp2, op=ALU.subtract)
                    nc.vector.tensor_scalar(vu, vu, BN_EPS, None, op0=ALU.add)
                    # rsqrt on VectorE: bit-trick seed + 2 Newton iterations
                    nc.vector.tensor_scalar(
                        sq.bitcast(i32), vu.bitcast(i32), 1, None,
                        op0=ALU.arith_shift_right,
                    )
                    nc.vector.tensor_scalar(
                        rr.bitcast(i32), sq.bitcast(i32), -1, 0x5F3759DF,
                        op0=ALU.mult, op1=ALU.add,
                    )
                    for _ in range(2):
                        nc.vector.tensor_tensor(sq, rr, rr, op=ALU.mult)
                        nc.vector.tensor_tensor(sq, sq, vu, op=ALU.mult)
                        nc.vector.tensor_scalar(
                            sq, sq, -0.5, 1.5, op0=ALU.mult, op1=ALU.add
                        )
                        nc.vector.tensor_tensor(rr, rr, sq, op=ALU.mult)
                    nc.vector.tensor_tensor(sv, g_sl, rr, op=ALU.mult)
                    nc.vector.tensor_tensor(tp2, mm, sv, op=ALU.mult)
                    nc.vector.tensor_tensor(tv, b_sl, tp2, op=ALU.subtract)

            def apply_group(li, out_h, j_lo, j_hi):
                with nc.named_scope(f"L{li}_apply{j_lo}"):
                    for j in range(j_lo, j_hi):
                        for b in range(NB):
                            nc.scalar.activation(
                                out_h[:, j, b],
                                out_h[:, j, b].bitcast(f32),
                                AF.Relu,
                                bias=t_vec[:, V_T + j : V_T + j + 1],
                                scale=t_vec[:, V_S + j : V_S + j + 1],
                            )

            def layer(li, Wcur, nkt, rhs, out_h, prep_next):
                """One hidden layer: matmuls + stats + allreduce + BN/ReLU apply."""
                with nc.named_scope(f"L{li}_mm"):
                    for j in range(NJ):
                        accs = [
                            pspool.tile(
                                [128, 512], f32, name=f"ps_l{li}_j{j}_b{b}", tag="ps"
                            )
                            for b in range(NB)
                        ]
                        for b in range(NB):
                            for k in range(nkt):
                                nc.tensor.matmul(
                                    accs[b][:],
                                    Wcur[:, k, j * 128 : (j + 1) * 128],
                                    rhs[:, k, b],
                                    start=(k == 0),
                                    stop=(k == nkt - 1),
                                )
                        for b in range(NB):
                            nc.vector.tensor_copy(out_h[:, j, b], accs[b][:])
                        for b in range(NB):
                            so = j * 24 + b * 6
                            nc.vector.bn_stats(
                                t_stats[:, so : so + 6], out_h[:, j, b].bitcast(f32)
                            )
                        # per-j partial: bn_aggr -> (mean, var); then E2 = var + mean^2
                        po = j * 2
                        nc.vector.bn_aggr(
                            t_part[:, po : po + 2],
                            t_stats[:, j * 24 : j * 24 + 24],
                        )
                        nc.vector.tensor_tensor(
                            t_vec[:, V_TMP + j : V_TMP + j + 1],
                            t_part[:, po : po + 1],
                            t_part[:, po : po + 1],
                            op=ALU.mult,
                        )
                        nc.vector.tensor_tensor(
                            t_part[:, po + 1 : po + 2],
                            t_vec[:, V_TMP + j : V_TMP + j + 1],
                            t_part[:, po + 1 : po + 2],
                            op=ALU.add,
                        )
                        if j == 1 and prep_next is not None:
                            prep_next()
                        if j == NJ - 3:
                            # early group: all-reduce features 0..(NJ-2);
                            # finishes during the j6/j7 tail
                            collective_group(li, 0, NJ - 2, "a")
                        if j == NJ - 2:
                            # emitted before j7's drains so the DVE stream
                            # does the s,t math as soon as the data is back
                            st_group(li, 0, NJ - 2, "a")
                            apply_group(li, out_h, 0, NJ - 2)
                # late group: the last two feature tiles
                collective_group(li, NJ - 2, NJ, "b")
                st_group(li, NJ - 2, NJ, "b")
                apply_group(li, out_h, NJ - 2, NJ)

            W2s = wpool.tile([128, NJ, H], f32r, name="W2s", tag="w")
            W3s = wpool.tile([128, NJ, H], f32r, name="W3s", tag="w")

            layer(1, W1s, KT1, xT_t, h1, lambda: prep_w(w2t, W2s, NJ, "w2"))
            layer(2, W2s, NJ, h1, h2, lambda: prep_w(w3t, W3s, NJ, "w3"))
            layer(3, W3s, NJ, h2, h3, None)

            # ---- head: 10-wide binarized linear + sigmoid ------------------
            with nc.named_scope("L4"):
                for b in range(NB):
                    acc = pspool.tile([D_OUT, 512], f32, name=f"ps_l4_b{b}", tag="ps")
                    for k in range(NJ):
                        nc.tensor.matmul(
                            acc[:],
                            W4s[:, k],
                            h3[:, k, b],
                            start=(k == 0),
                            stop=(k == NJ - 1),
                        )
                    osb = outpool.tile([D_OUT, 512], f32, name=f"osb{b}", tag="osb")
                    nc.scalar.activation(osb[:], acc[:], AF.Sigmoid)
                    nc.sync.dma_start(outT[:, b * 512 : (b + 1) * 512], osb[:])

    nc.compile()
    return nc


_NC = None
_LAST_RESULTS = None


def _get_nc():
    global _NC
    if _NC is None:
        nc = bacc.Bacc(
            "TRN2", target_bir_lowering=False, debug=False, num_devices=N_CORES
        )
        build(nc)
        _NC = nc
    return _NC


def kernel(**inputs):
    x = np.ascontiguousarray(inputs["x"], dtype=np.float32)
    w1 = np.asarray(inputs["w1"], dtype=np.float32)
    w2 = np.asarray(inputs["w2"], dtype=np.float32)
    w3 = np.asarray(inputs["w3"], dtype=np.float32)
    w4 = np.asarray(inputs["w4"], dtype=np.float32)
    gb = np.stack(
        [
            np.asarray(inputs[n], dtype=np.float32)
            for n in ("g1", "b1", "g2", "b2", "g3", "b3")
        ]
    )  # [6, 1024]

    w1t = np.zeros((D_IN_PAD, H), np.float32)
    w1t[:D_IN] = w1.T
    w2t = np.ascontiguousarray(w2.T)
    w3t = np.ascontiguousarray(w3.T)
    w4t = np.ascontiguousarray(w4.T)
    gbp = np.ascontiguousarray(gb.reshape(6, 8, 128).transpose(2, 0, 1))  # [128,6,8]

    nc = _get_nc()
    in_maps = []
    for c in range(N_CORES):
        xs = np.zeros((D_IN_PAD, BPC), np.float32)
        xs[:D_IN] = x[c * BPC : (c + 1) * BPC].T
        xs = np.ascontiguousarray(
            xs.reshape(KT1, 128, NB, 512).transpose(0, 2, 1, 3)
        )
        in_maps.append(
            {"xT": xs, "w1t": w1t, "w2t": w2t, "w3t": w3t, "w4t": w4t, "gbp": gbp}
        )

    res = run_bass_kernel_spmd(nc, in_maps, core_ids=list(range(N_CORES)))
    global _LAST_RESULTS
    _LAST_RESULTS = res
    out = np.empty((B_TOT, D_OUT), dtype=np.float32)
    for c in range(N_CORES):
        out[c * BPC : (c + 1) * BPC] = res.results[c]["outT"].T
    return out


# revision 17
# speedup vs baseline: 1.7273x; 1.7273x over previous
"""Binarized MLP forward (BinaryConnect, training-mode BatchNorm) on 8 TRN2 cores.

Strategy: data-parallel over the batch (16384 -> 8 x 2048), weights replicated.
All activations kept TRANSPOSED on device ([features, batch]) so that
 - matmuls use binarized weights as the stationary operand,
 - BatchNorm stats are free-axis reductions (bn_stats on VectorE),
 - BN apply + ReLU is a single per-partition scale/bias activation on ScalarE.
Per-feature batch statistics are all-reduced across the 8 cores (8 KB/layer),
split into an early group (features 0..895, overlapped with the layer tail)
and a late group (last 128 features) to keep the boundary short.
Matmuls run in float32r (full PE rate at N=512; binarized +-1 weights exact).
"""
import os
import numpy as np

import concourse.bass as bass
import concourse.bacc as bacc
import concourse.tile as tile
from concourse.tile_rust import add_dep_helper
import concourse.mybir as mybir
from concourse.bass_utils import run_bass_kernel_spmd

N_CORES = 8
B_TOT = 16384
BPC = B_TOT // N_CORES  # 2048 batch rows per core
NB = BPC // 512  # 4 free-dim tiles of 512
D_IN, H, D_OUT = 784, 1024, 10
D_IN_PAD = 896  # pad 784 -> 7 full k-tiles of 128
KT1 = D_IN_PAD // 128
NJ = H // 128  # 8 feature tiles per hidden layer
BN_EPS = 1e-5

f32 = mybir.dt.float32
f32r = mybir.dt.float32r
i32 = mybir.dt.int32
AF = mybir.ActivationFunctionType
ALU = mybir.AluOpType

# t_vec scratch layout (free-dim float offsets)
V_S = 0       # BN scale per feature (8)
V_T = 8       # BN shift per feature (8)
V_M = 16      # mean
V_E2 = 24
V_VU = 32     # var, then var+eps
V_SQ = 40     # sqrt(var+eps)
V_R = 48      # rsqrt
V_TMP = 56
V_TMP2 = 64


def build(nc):
    xT = nc.dram_tensor("xT", [KT1, NB, 128, 512], f32r, kind="ExternalInput")
    w1t = nc.dram_tensor("w1t", [D_IN_PAD, H], f32, kind="ExternalInput")
    w2t = nc.dram_tensor("w2t", [H, H], f32, kind="ExternalInput")
    w3t = nc.dram_tensor("w3t", [H, H], f32, kind="ExternalInput")
    w4t = nc.dram_tensor("w4t", [H, D_OUT], f32, kind="ExternalInput")
    gbp = nc.dram_tensor("gbp", [128, 6, 8], f32, kind="ExternalInput")
    outT = nc.dram_tensor("outT", [D_OUT, BPC], f32, kind="ExternalOutput")

    rg = [list(range(N_CORES))]

    with tile.TileContext(nc) as tc:
        with (
            tc.tile_pool(name="hp", bufs=2) as hpool,
            tc.tile_pool(name="wp", bufs=2) as wpool,
            tc.tile_pool(name="w4p", bufs=1) as w4pool,
            tc.tile_pool(name="stg", bufs=4) as stgpool,
            tc.tile_pool(name="outp", bufs=2) as outpool,
            tc.tile_pool(name="msc", bufs=1) as mpool,
            tc.tile_pool(name="ps", bufs=8, space="PSUM") as pspool,
            tc.tile_pool(name="dram", bufs=1, space="DRAM") as dpool,
        ):
            t_stats = mpool.tile([128, 192], f32, name="t_stats")
            t_part = mpool.tile([128, 16], f32, name="t_part")
            t_gst = mpool.tile([128, 16], f32, name="t_gst")
            t_vec = mpool.tile([128, 72], f32, name="t_vec")
            t_gb = mpool.tile([128, 48], f32, name="t_gb")

            # --- warmup collective: absorb first-call ncfw/algorithm cost.
            # Pure DRAM->DRAM with unread output: zero coupling with the
            # compute DMA queues or SBUF dependency tracking.
            with nc.named_scope("warmup_ar"):
                win = dpool.tile([128, 2], f32, name="warm_in")
                wout = dpool.tile([128, 2], f32, name="warm_out", addr_space="Shared")
                nc.gpsimd.collective_compute(
                    "AllReduce", ALU.add, replica_groups=rg,
                    ins=[win[:].opt()], outs=[wout[:].opt()],
                )

            def prep_w(wt_dram, Wtile, nkt, tag_suffix, after=None):
                """DMA raw transposed weights into staging (half-tiles for a
                finer DMA/Sign pipeline), binarize (Sign). The first Sign is
                order-pinned after `after` so the scheduler cannot hoist the
                sign burst into the previous layer's boundary ACT window."""
                after_inst = [after]
                for k in range(nkt):
                    for hh in range(2):
                        stg = stgpool.tile(
                            [128, H // 2], f32, name=f"stg_{tag_suffix}_{k}_{hh}", tag="stg"
                        )
                        nc.sync.dma_start(
                            stg[:],
                            wt_dram[k * 128 : (k + 1) * 128, hh * 512 : (hh + 1) * 512],
                        )
                        si = nc.scalar.activation(
                            Wtile[:, k, hh * 512 : (hh + 1) * 512], stg[:], AF.Sign
                        )
                        if after_inst[0] is not None:
                            add_dep_helper(
                                si.ins, after_inst[0], False,
                                "keep boundary ACT ops ahead of weight signs",
                            )
                            after_inst[0] = None

            # --- input loads, in first-consumer order: the layer-1 j0 column
            # needs W1s[k] and xT[b=0, k] for every k first.
            xT_t = hpool.tile([128, KT1, NB, 512], f32r, name="xT_t", tag="h")
            W1s = wpool.tile([128, KT1, H], f32r, name="W1s", tag="w")
            with nc.named_scope("w1prep"):
                for k in range(KT1):
                    for hh in range(2):
                        stg = stgpool.tile(
                            [128, H // 2], f32, name=f"stg_w1_{k}_{hh}", tag="stg"
                        )
                        nc.sync.dma_start(
                            stg[:],
                            w1t[k * 128 : (k + 1) * 128, hh * 512 : (hh + 1) * 512],
                        )
                        nc.scalar.activation(
                            W1s[:, k, hh * 512 : (hh + 1) * 512], stg[:], AF.Sign
                        )
                    nc.sync.dma_start(xT_t[:, k, 0], xT[k, 0])
            with nc.named_scope("xload"):
                for b in range(1, NB):
                    for k in range(KT1):
                        nc.sync.dma_start(xT_t[:, k, b], xT[k, b])
            nc.sync.dma_start(t_gb[:], gbp[:].rearrange("p a b -> p (a b)"))

            W4s = w4pool.tile([128, NJ, D_OUT], f32r, name="W4s")
            with nc.named_scope("w4prep"):
                stg4 = stgpool.tile([128, NJ, D_OUT], f32, name="stg_w4", tag="stg")
                nc.sync.dma_start(
                    stg4[:], w4t[:].rearrange("(kt p) c -> p kt c", p=128)
                )
                nc.scalar.activation(
                    W4s[:].rearrange("p a b -> p (a b)"),
                    stg4[:].rearrange("p a b -> p (a b)"),
                    AF.Sign,
                )

            h1 = hpool.tile([128, NJ, NB, 512], f32r, name="h1", tag="h")
            h2 = hpool.tile([128, NJ, NB, 512], f32r, name="h2", tag="h")
            h3 = hpool.tile([128, NJ, NB, 512], f32r, name="h3", tag="h")

            def collective_group(li, j_lo, j_hi, gtag):
                """All-reduce partial stats for feature tiles [j_lo, j_hi)."""
                n = (j_hi - j_lo) * 2
                with nc.named_scope(f"L{li}_ar{gtag}"):
                    cin = dpool.tile([128, n], f32, name=f"cin{li}{gtag}")
                    cout = dpool.tile(
                        [128, n], f32, name=f"cout{li}{gtag}", addr_space="Shared"
                    )
                    nc.gpsimd.dma_start(
                        cin[:], t_part[:, j_lo * 2 : j_hi * 2]
                    )
                    nc.gpsimd.collective_compute(
                        "AllReduce", ALU.add, replica_groups=rg,
                        ins=[cin[:].opt()], outs=[cout[:].opt()],
                    )
                    nc.gpsimd.dma_start(t_gst[:, j_lo * 2 : j_hi * 2], cout[:])

            def st_group(li, j_lo, j_hi, gtag):
                """s = g*rsqrt(v+eps), t = b - m*s for feature tiles [j_lo, j_hi)."""
                with nc.named_scope(f"L{li}_st{gtag}"):
                    gview = t_gst[:, j_lo * 2 : j_hi * 2].rearrange(
                        "p (j c) -> p j c", c=2
                    )
                    mm = t_vec[:, V_M + j_lo : V_M + j_hi]
                    e2 = t_vec[:, V_E2 + j_lo : V_E2 + j_hi]
                    vu = t_vec[:, V_VU + j_lo : V_VU + j_hi]
                    sq = t_vec[:, V_SQ + j_lo : V_SQ + j_hi]
                    rr = t_vec[:, V_R + j_lo : V_R + j_hi]
                    tp2 = t_vec[:, V_TMP2 + j_lo : V_TMP2 + j_hi]
                    sv = t_vec[:, V_S + j_lo : V_S + j_hi]
                    tv = t_vec[:, V_T + j_lo : V_T + j_hi]
                    g_sl = t_gb[:, (li - 1) * 16 + j_lo : (li - 1) * 16 + j_hi]
                    b_sl = t_gb[:, (li - 1) * 16 + 8 + j_lo : (li - 1) * 16 + 8 + j_hi]
                    nc.vector.tensor_scalar(mm, gview[:, :, 0], 1.0 / N_CORES, None, op0=ALU.mult)
                    nc.vector.tensor_scalar(e2, gview[:, :, 1], 1.0 / N_CORES, None, op0=ALU.mult)
                    nc.vector.tensor_tensor(tp2, mm, mm, op=ALU.mult)
                    nc.vector.tensor_tensor(vu, e2, tp2, op=ALU.subtract)
                    nc.vector.tensor_scalar(vu, vu, BN_EPS, None, op0=ALU.add)
                    nc.scalar.activation(sq, vu, AF.Sqrt)
                    nc.vector.reciprocal(rr, sq)
                    nc.vector.tensor_tensor(sv, g_sl, rr, op=ALU.mult)
                    nc.vector.tensor_tensor(tp2, mm, sv, op=ALU.mult)
                    nc.vector.tensor_tensor(tv, b_sl, tp2, op=ALU.subtract)

            def apply_group(li, out_h, j_lo, j_hi):
                last = None
                with nc.named_scope(f"L{li}_apply{j_lo}"):
                    for j in range(j_lo, j_hi):
                        for b in range(NB):
                            last = nc.scalar.activation(
                                out_h[:, j, b],
                                out_h[:, j, b].bitcast(f32),
                                AF.Relu,
                                bias=t_vec[:, V_T + j : V_T + j + 1],
                                scale=t_vec[:, V_S + j : V_S + j + 1],
                            )
                return last

            def layer(li, Wcur, nkt, rhs, out_h, prep_next):
                """One hidden layer: matmuls + stats + allreduce + BN/ReLU apply."""
                with nc.named_scope(f"L{li}_mm"):
                    for j in range(NJ):
                        accs = [
                            pspool.tile(
                                [128, 512], f32, name=f"ps_l{li}_j{j}_b{b}", tag="ps"
                            )
                            for b in range(NB)
                        ]
                        for b in range(NB):
                            for k in range(nkt):
                                nc.tensor.matmul(
                                    accs[b][:],
                                    Wcur[:, k, j * 128 : (j + 1) * 128],
                                    rhs[:, k, b],
                                    start=(k == 0),
                                    stop=(k == nkt - 1),
                                )
                        for b in range(NB):
                            nc.vector.tensor_copy(out_h[:, j, b], accs[b][:])
                        for b in range(NB):
                            so = j * 24 + b * 6
                            nc.vector.bn_stats(
                                t_stats[:, so : so + 6], out_h[:, j, b].bitcast(f32)
                            )
                        # per-j partial: bn_aggr -> (mean, var); then E2 = var + mean^2
                        po = j * 2
                        nc.vector.bn_aggr(
                            t_part[:, po : po + 2],
                            t_stats[:, j * 24 : j * 24 + 24],
                        )
                        nc.vector.tensor_tensor(
                            t_vec[:, V_TMP + j : V_TMP + j + 1],
                            t_part[:, po : po + 1],
                            t_part[:, po : po + 1],
                            op=ALU.mult,
                        )
                        nc.vector.tensor_tensor(
                            t_part[:, po + 1 : po + 2],
                            t_vec[:, V_TMP + j : V_TMP + j + 1],
                            t_part[:, po + 1 : po + 2],
                            op=ALU.add,
                        )
                        if j == 1 and prep_next is not None:
                            prep_next(prev_apply[0])
                        if j == NJ - 3:
                            # early group: all-reduce features 0..(NJ-2);
                            # finishes during the j6/j7 tail
                            collective_group(li, 0, NJ - 2, "a")
                        if j == NJ - 2:
                            # emitted before j7's drains so the DVE stream
                            # does the s,t math as soon as the data is back
                            st_group(li, 0, NJ - 2, "a")
                            apply_group(li, out_h, 0, NJ - 2)
                # late group: the last two feature tiles
                collective_group(li, NJ - 2, NJ, "b")
                st_group(li, NJ - 2, NJ, "b")
                return apply_group(li, out_h, NJ - 2, NJ)

            W2s = wpool.tile([128, NJ, H], f32r, name="W2s", tag="w")
            W3s = wpool.tile([128, NJ, H], f32r, name="W3s", tag="w")

            prev_apply = [None]
            a1 = layer(1, W1s, KT1, xT_t, h1,
                       lambda after: prep_w(w2t, W2s, NJ, "w2", after))
            prev_apply[0] = a1.ins if a1 is not None else None
            a2 = layer(2, W2s, NJ, h1, h2,
                       lambda after: prep_w(w3t, W3s, NJ, "w3", after))
            prev_apply[0] = a2.ins if a2 is not None else None
            layer(3, W3s, NJ, h2, h3, None)

            # ---- head: 10-wide binarized linear + sigmoid ------------------
            with nc.named_scope("L4"):
                for b in range(NB):
                    acc = pspool.tile([D_OUT, 512], f32, name=f"ps_l4_b{b}", tag="ps")
                    for k in range(NJ):
                        nc.tensor.matmul(
                            acc[:],
                            W4s[:, k],
                            h3[:, k, b],
                            start=(k == 0),
                            stop=(k == NJ - 1),
                        )
                    osb = outpool.tile([D_OUT, 512], f32, name=f"osb{b}", tag="osb")
                    nc.scalar.activation(osb[:], acc[:], AF.Sigmoid)
                    nc.sync.dma_start(outT[:, b * 512 : (b + 1) * 512], osb[:])

    nc.compile()
    return nc


_NC = None
_LAST_RESULTS = None


def _get_nc():
    global _NC
    if _NC is None:
        nc = bacc.Bacc(
            "TRN2", target_bir_lowering=False, debug=False, num_devices=N_CORES
        )
        build(nc)
        _NC = nc
    return _NC


def kernel(**inputs):
    x = np.ascontiguousarray(inputs["x"], dtype=np.float32)
    w1 = np.asarray(inputs["w1"], dtype=np.float32)
    w2 = np.asarray(inputs["w2"], dtype=np.float32)
    w3 = np.asarray(inputs["w3"], dtype=np.float32)
    w4 = np.asarray(inputs["w4"], dtype=np.float32)
    gb = np.stack(
        [
            np.asarray(inputs[n], dtype=np.float32)
            for n in ("g1", "b1", "g2", "b2", "g3", "b3")
        ]
    )  # [6, 1024]

    w1t = np.zeros((D_IN_PAD, H), np.float32)
    w1t[:D_IN] = w1.T
    w2t = np.ascontiguousarray(w2.T)
    w3t = np.ascontiguousarray(w3.T)
    w4t = np.ascontiguousarray(w4.T)
    gbp = np.ascontiguousarray(gb.reshape(6, 8, 128).transpose(2, 0, 1))  # [128,6,8]

    nc = _get_nc()
    in_maps = []
    for c in range(N_CORES):
        xs = np.zeros((D_IN_PAD, BPC), np.float32)
        xs[:D_IN] = x[c * BPC : (c + 1) * BPC].T
        xs = np.ascontiguousarray(
            xs.reshape(KT1, 128, NB, 512).transpose(0, 2, 1, 3)
        )
        in_maps.append(
            {"xT": xs, "w1t": w1t, "w2t": w2t, "w3t": w3t, "w4t": w4t, "gbp": gbp}
        )

    res = run_bass_kernel_spmd(nc, in_maps, core_ids=list(range(N_CORES)))
    global _LAST_RESULTS
    _LAST_RESULTS = res
    out = np.empty((B_TOT, D_OUT), dtype=np.float32)
    for c in range(N_CORES):
        out[c * BPC : (c + 1) * BPC] = res.results[c]["outT"].T
    return out


# revision 18
# speedup vs baseline: 1.7878x; 1.0351x over previous
"""Binarized MLP forward (BinaryConnect, training-mode BatchNorm) on 8 TRN2 cores.

Strategy: data-parallel over the batch (16384 -> 8 x 2048), weights replicated.
All activations kept TRANSPOSED on device ([features, batch]) so that
 - matmuls use binarized weights as the stationary operand,
 - BatchNorm stats are free-axis reductions (bn_stats on VectorE),
 - BN apply + ReLU is a single per-partition scale/bias activation on ScalarE.
Per-feature batch statistics are all-reduced across the 8 cores (8 KB/layer),
split into an early group (features 0..895, overlapped with the layer tail)
and a late group (last 128 features) to keep the boundary short.
Matmuls run in float32r (full PE rate at N=512; binarized +-1 weights exact).
"""
import os
import numpy as np

import concourse.bass as bass
import concourse.bacc as bacc
import concourse.tile as tile
from concourse.tile_rust import add_dep_helper
import concourse.mybir as mybir
from concourse.bass_utils import run_bass_kernel_spmd

N_CORES = 8
B_TOT = 16384
BPC = B_TOT // N_CORES  # 2048 batch rows per core
NB = BPC // 512  # 4 free-dim tiles of 512
D_IN, H, D_OUT = 784, 1024, 10
D_IN_PAD = 896  # pad 784 -> 7 full k-tiles of 128
KT1 = D_IN_PAD // 128
NJ = H // 128  # 8 feature tiles per hidden layer
BN_EPS = 1e-5

f32 = mybir.dt.float32
f32r = mybir.dt.float32r
i32 = mybir.dt.int32
bf16 = mybir.dt.bfloat16
AF = mybir.ActivationFunctionType
ALU = mybir.AluOpType

# t_vec scratch layout (free-dim float offsets)
V_S = 0       # BN scale per feature (8)
V_T = 8       # BN shift per feature (8)
V_M = 16      # mean
V_E2 = 24
V_VU = 32     # var, then var+eps
V_SQ = 40     # sqrt(var+eps)
V_R = 48      # rsqrt
V_TMP = 56
V_TMP2 = 64


def build(nc):
    xT = nc.dram_tensor("xT", [KT1, NB, 128, 512], f32r, kind="ExternalInput")
    w1t = nc.dram_tensor("w1t", [D_IN_PAD, H], bf16, kind="ExternalInput")
    w2t = nc.dram_tensor("w2t", [H, H], bf16, kind="ExternalInput")
    w3t = nc.dram_tensor("w3t", [H, H], bf16, kind="ExternalInput")
    w4t = nc.dram_tensor("w4t", [H, D_OUT], bf16, kind="ExternalInput")
    gbp = nc.dram_tensor("gbp", [128, 6, 8], f32, kind="ExternalInput")
    outT = nc.dram_tensor("outT", [D_OUT, BPC], f32, kind="ExternalOutput")

    rg = [list(range(N_CORES))]

    with tile.TileContext(nc) as tc:
        with (
            tc.tile_pool(name="hp", bufs=2) as hpool,
            tc.tile_pool(name="wp", bufs=2) as wpool,
            tc.tile_pool(name="w4p", bufs=1) as w4pool,
            tc.tile_pool(name="stg", bufs=4) as stgpool,
            tc.tile_pool(name="outp", bufs=2) as outpool,
            tc.tile_pool(name="msc", bufs=1) as mpool,
            tc.tile_pool(name="ps", bufs=8, space="PSUM") as pspool,
            tc.tile_pool(name="dram", bufs=1, space="DRAM") as dpool,
        ):
            t_stats = mpool.tile([128, 192], f32, name="t_stats")
            t_part = mpool.tile([128, 16], f32, name="t_part")
            t_gst = mpool.tile([128, 16], f32, name="t_gst")
            t_vec = mpool.tile([128, 72], f32, name="t_vec")
            t_gb = mpool.tile([128, 48], f32, name="t_gb")

            # --- warmup collective: absorb first-call ncfw/algorithm cost.
            # Pure DRAM->DRAM with unread output: zero coupling with the
            # compute DMA queues or SBUF dependency tracking.
            with nc.named_scope("warmup_ar"):
                win = dpool.tile([128, 2], f32, name="warm_in")
                wout = dpool.tile([128, 2], f32, name="warm_out", addr_space="Shared")
                nc.gpsimd.collective_compute(
                    "AllReduce", ALU.add, replica_groups=rg,
                    ins=[win[:].opt()], outs=[wout[:].opt()],
                )

            def prep_w(wt_dram, Wtile, nkt, tag_suffix, after=None):
                """DMA raw transposed weights into staging (half-tiles for a
                finer DMA/Sign pipeline), binarize (Sign). The first Sign is
                order-pinned after `after` so the scheduler cannot hoist the
                sign burst into the previous layer's boundary ACT window."""
                after_inst = [after]
                for k in range(nkt):
                    for hh in range(2):
                        stg = stgpool.tile(
                            [128, H // 2], bf16, name=f"stg_{tag_suffix}_{k}_{hh}", tag="stg"
                        )
                        nc.sync.dma_start(
                            stg[:],
                            wt_dram[k * 128 : (k + 1) * 128, hh * 512 : (hh + 1) * 512],
                        )
                        si = nc.scalar.activation(
                            Wtile[:, k, hh * 512 : (hh + 1) * 512], stg[:], AF.Sign
                        )
                        if after_inst[0] is not None:
                            add_dep_helper(
                                si.ins, after_inst[0], False,
                                "keep boundary ACT ops ahead of weight signs",
                            )
                            after_inst[0] = None

            # --- input loads, in first-consumer order: the layer-1 j0 column
            # needs W1s[k] and xT[b=0, k] for every k first.
            xT_t = hpool.tile([128, KT1, NB, 512], f32r, name="xT_t", tag="h")
            W1s = wpool.tile([128, KT1, H], f32r, name="W1s", tag="w")
            with nc.named_scope("w1prep"):
                for k in range(KT1):
                    for hh in range(2):
                        stg = stgpool.tile(
                            [128, H // 2], bf16, name=f"stg_w1_{k}_{hh}", tag="stg"
                        )
                        nc.sync.dma_start(
                            stg[:],
                            w1t[k * 128 : (k + 1) * 128, hh * 512 : (hh + 1) * 512],
                        )
                        nc.scalar.activation(
                            W1s[:, k, hh * 512 : (hh + 1) * 512], stg[:], AF.Sign
                        )
                    nc.sync.dma_start(xT_t[:, k, 0], xT[k, 0])
            with nc.named_scope("xload"):
                for b in range(1, NB):
                    for k in range(KT1):
                        nc.sync.dma_start(xT_t[:, k, b], xT[k, b])
            nc.sync.dma_start(t_gb[:], gbp[:].rearrange("p a b -> p (a b)"))

            W4s = w4pool.tile([128, NJ, D_OUT], f32r, name="W4s")
            with nc.named_scope("w4prep"):
                stg4 = stgpool.tile([128, NJ, D_OUT], bf16, name="stg_w4", tag="stg")
                nc.sync.dma_start(
                    stg4[:], w4t[:].rearrange("(kt p) c -> p kt c", p=128)
                )
                nc.scalar.activation(
                    W4s[:].rearrange("p a b -> p (a b)"),
                    stg4[:].rearrange("p a b -> p (a b)"),
                    AF.Sign,
                )

            h1 = hpool.tile([128, NJ, NB, 512], f32r, name="h1", tag="h")
            h2 = hpool.tile([128, NJ, NB, 512], f32r, name="h2", tag="h")
            h3 = hpool.tile([128, NJ, NB, 512], f32r, name="h3", tag="h")

            def collective_group(li, j_lo, j_hi, gtag):
                """All-reduce partial stats for feature tiles [j_lo, j_hi)."""
                n = (j_hi - j_lo) * 2
                with nc.named_scope(f"L{li}_ar{gtag}"):
                    cin = dpool.tile([128, n], f32, name=f"cin{li}{gtag}")
                    cout = dpool.tile(
                        [128, n], f32, name=f"cout{li}{gtag}", addr_space="Shared"
                    )
                    nc.gpsimd.dma_start(
                        cin[:], t_part[:, j_lo * 2 : j_hi * 2]
                    )
                    nc.gpsimd.collective_compute(
                        "AllReduce", ALU.add, replica_groups=rg,
                        ins=[cin[:].opt()], outs=[cout[:].opt()],
                    )
                    nc.gpsimd.dma_start(t_gst[:, j_lo * 2 : j_hi * 2], cout[:])

            def st_group(li, j_lo, j_hi, gtag):
                """s = g*rsqrt(v+eps), t = b - m*s for feature tiles [j_lo, j_hi)."""
                with nc.named_scope(f"L{li}_st{gtag}"):
                    gview = t_gst[:, j_lo * 2 : j_hi * 2].rearrange(
                        "p (j c) -> p j c", c=2
                    )
                    mm = t_vec[:, V_M + j_lo : V_M + j_hi]
                    e2 = t_vec[:, V_E2 + j_lo : V_E2 + j_hi]
                    vu = t_vec[:, V_VU + j_lo : V_VU + j_hi]
                    sq = t_vec[:, V_SQ + j_lo : V_SQ + j_hi]
                    rr = t_vec[:, V_R + j_lo : V_R + j_hi]
                    tp2 = t_vec[:, V_TMP2 + j_lo : V_TMP2 + j_hi]
                    sv = t_vec[:, V_S + j_lo : V_S + j_hi]
                    tv = t_vec[:, V_T + j_lo : V_T + j_hi]
                    g_sl = t_gb[:, (li - 1) * 16 + j_lo : (li - 1) * 16 + j_hi]
                    b_sl = t_gb[:, (li - 1) * 16 + 8 + j_lo : (li - 1) * 16 + 8 + j_hi]
                    nc.vector.tensor_scalar(mm, gview[:, :, 0], 1.0 / N_CORES, None, op0=ALU.mult)
                    nc.vector.tensor_scalar(e2, gview[:, :, 1], 1.0 / N_CORES, None, op0=ALU.mult)
                    nc.vector.tensor_tensor(tp2, mm, mm, op=ALU.mult)
                    nc.vector.tensor_tensor(vu, e2, tp2, op=ALU.subtract)
                    nc.vector.tensor_scalar(vu, vu, BN_EPS, None, op0=ALU.add)
                    nc.scalar.activation(sq, vu, AF.Sqrt)
                    nc.vector.reciprocal(rr, sq)
                    nc.vector.tensor_tensor(sv, g_sl, rr, op=ALU.mult)
                    nc.vector.tensor_tensor(tp2, mm, sv, op=ALU.mult)
                    nc.vector.tensor_tensor(tv, b_sl, tp2, op=ALU.subtract)

            def apply_group(li, out_h, j_lo, j_hi):
                last = None
                with nc.named_scope(f"L{li}_apply{j_lo}"):
                    for j in range(j_lo, j_hi):
                        for b in range(NB):
                            last = nc.scalar.activation(
                                out_h[:, j, b],
                                out_h[:, j, b].bitcast(f32),
                                AF.Relu,
                                bias=t_vec[:, V_T + j : V_T + j + 1],
                                scale=t_vec[:, V_S + j : V_S + j + 1],
                            )
                return last

            def layer(li, Wcur, nkt, rhs, out_h, prep_next):
                """One hidden layer: matmuls + stats + allreduce + BN/ReLU apply."""
                with nc.named_scope(f"L{li}_mm"):
                    for j in range(NJ):
                        accs = [
                            pspool.tile(
                                [128, 512], f32, name=f"ps_l{li}_j{j}_b{b}", tag="ps"
                            )
                            for b in range(NB)
                        ]
                        for b in range(NB):
                            for k in range(nkt):
                                nc.tensor.matmul(
                                    accs[b][:],
                                    Wcur[:, k, j * 128 : (j + 1) * 128],
                                    rhs[:, k, b],
                                    start=(k == 0),
                                    stop=(k == nkt - 1),
                                )
                        for b in range(NB):
                            nc.vector.tensor_copy(out_h[:, j, b], accs[b][:])
                        for b in range(NB):
                            so = j * 24 + b * 6
                            nc.vector.bn_stats(
                                t_stats[:, so : so + 6], out_h[:, j, b].bitcast(f32)
                            )
                        # per-j partial: bn_aggr -> (mean, var); then E2 = var + mean^2
                        po = j * 2
                        nc.vector.bn_aggr(
                            t_part[:, po : po + 2],
                            t_stats[:, j * 24 : j * 24 + 24],
                        )
                        nc.vector.tensor_tensor(
                            t_vec[:, V_TMP + j : V_TMP + j + 1],
                            t_part[:, po : po + 1],
                            t_part[:, po : po + 1],
                            op=ALU.mult,
                        )
                        nc.vector.tensor_tensor(
                            t_part[:, po + 1 : po + 2],
                            t_vec[:, V_TMP + j : V_TMP + j + 1],
                            t_part[:, po + 1 : po + 2],
                            op=ALU.add,
                        )
                        if j == 1 and prep_next is not None:
                            prep_next(prev_apply[0])
                        if j == NJ - 3:
                            # early group: all-reduce features 0..(NJ-2);
                            # finishes during the j6/j7 tail
                            collective_group(li, 0, NJ - 2, "a")
                        if j == NJ - 2:
                            # emitted before j7's drains so the DVE stream
                            # does the s,t math as soon as the data is back
                            st_group(li, 0, NJ - 2, "a")
                            apply_group(li, out_h, 0, NJ - 2)
                # late group: the last two feature tiles
                collective_group(li, NJ - 2, NJ, "b")
                st_group(li, NJ - 2, NJ, "b")
                return apply_group(li, out_h, NJ - 2, NJ)

            W2s = wpool.tile([128, NJ, H], f32r, name="W2s", tag="w")
            W3s = wpool.tile([128, NJ, H], f32r, name="W3s", tag="w")

            prev_apply = [None]
            a1 = layer(1, W1s, KT1, xT_t, h1,
                       lambda after: prep_w(w2t, W2s, NJ, "w2", after))
            prev_apply[0] = a1.ins if a1 is not None else None
            a2 = layer(2, W2s, NJ, h1, h2,
                       lambda after: prep_w(w3t, W3s, NJ, "w3", after))
            prev_apply[0] = a2.ins if a2 is not None else None
            layer(3, W3s, NJ, h2, h3, None)

            # ---- head: 10-wide binarized linear + sigmoid ------------------
            with nc.named_scope("L4"):
                for b in range(NB):
                    acc = pspool.tile([D_OUT, 512], f32, name=f"ps_l4_b{b}", tag="ps")
                    for k in range(NJ):
                        nc.tensor.matmul(
                            acc[:],
                            W4s[:, k],
                            h3[:, k, b],
                            start=(k == 0),
                            stop=(k == NJ - 1),
                        )
                    osb = outpool.tile([D_OUT, 512], f32, name=f"osb{b}", tag="osb")
                    nc.scalar.activation(osb[:], acc[:], AF.Sigmoid)
                    nc.sync.dma_start(outT[:, b * 512 : (b + 1) * 512], osb[:])

    nc.compile()
    return nc


_NC = None
_LAST_RESULTS = None


def _get_nc():
    global _NC
    if _NC is None:
        nc = bacc.Bacc(
            "TRN2", target_bir_lowering=False, debug=False, num_devices=N_CORES
        )
        build(nc)
        _NC = nc
    return _NC


def kernel(**inputs):
    x = np.ascontiguousarray(inputs["x"], dtype=np.float32)
    w1 = np.asarray(inputs["w1"], dtype=np.float32)
    w2 = np.asarray(inputs["w2"], dtype=np.float32)
    w3 = np.asarray(inputs["w3"], dtype=np.float32)
    w4 = np.asarray(inputs["w4"], dtype=np.float32)
    gb = np.stack(
        [
            np.asarray(inputs[n], dtype=np.float32)
            for n in ("g1", "b1", "g2", "b2", "g3", "b3")
        ]
    )  # [6, 1024]

    import ml_dtypes

    bf = ml_dtypes.bfloat16
    w1t = np.zeros((D_IN_PAD, H), bf)
    w1t[:D_IN] = w1.T.astype(bf)
    w2t = np.ascontiguousarray(w2.T.astype(bf))
    w3t = np.ascontiguousarray(w3.T.astype(bf))
    w4t = np.ascontiguousarray(w4.T.astype(bf))
    gbp = np.ascontiguousarray(gb.reshape(6, 8, 128).transpose(2, 0, 1))  # [128,6,8]

    nc = _get_nc()
    in_maps = []
    for c in range(N_CORES):
        xs = np.zeros((D_IN_PAD, BPC), np.float32)
        xs[:D_IN] = x[c * BPC : (c + 1) * BPC].T
        xs = np.ascontiguousarray(
            xs.reshape(KT1, 128, NB, 512).transpose(0, 2, 1, 3)
        )
        in_maps.append(
            {"xT": xs, "w1t": w1t, "w2t": w2t, "w3t": w3t, "w4t": w4t, "gbp": gbp}
        )

    res = run_bass_kernel_spmd(nc, in_maps, core_ids=list(range(N_CORES)))
    global _LAST_RESULTS
    _LAST_RESULTS = res
    out = np.empty((B_TOT, D_OUT), dtype=np.float32)
    for c in range(N_CORES):
        out[c * BPC : (c + 1) * BPC] = res.results[c]["outT"].T
    return out


# revision 19
# speedup vs baseline: 1.9328x; 1.0811x over previous
"""Binarized MLP forward (BinaryConnect, training-mode BatchNorm) on 8 TRN2 cores.

Strategy: data-parallel over the batch (16384 -> 8 x 2048), weights replicated.
All activations kept TRANSPOSED on device ([features, batch]) so that
 - matmuls use binarized weights as the stationary operand,
 - BatchNorm stats are free-axis reductions (bn_stats on VectorE),
 - BN apply + ReLU is a single per-partition scale/bias activation on ScalarE.
Per-feature batch statistics are all-reduced across the 8 cores (8 KB/layer),
split into an early group (features 0..895, overlapped with the layer tail)
and a late group (last 128 features) to keep the boundary short.
Matmuls run in float32r (full PE rate at N=512; binarized +-1 weights exact).
"""
import os
import numpy as np

import concourse.bass as bass
import concourse.bacc as bacc
import concourse.tile as tile
from concourse.tile_rust import add_dep_helper
import concourse.mybir as mybir
from concourse.bass_utils import run_bass_kernel_spmd

N_CORES = 8
B_TOT = 16384
BPC = B_TOT // N_CORES  # 2048 batch rows per core
NB = BPC // 512  # 4 free-dim tiles of 512
D_IN, H, D_OUT = 784, 1024, 10
D_IN_PAD = 896  # pad 784 -> 7 full k-tiles of 128
KT1 = D_IN_PAD // 128
NJ = H // 128  # 8 feature tiles per hidden layer
BN_EPS = 1e-5

f32 = mybir.dt.float32
f32r = mybir.dt.float32r
i32 = mybir.dt.int32
bf16 = mybir.dt.bfloat16
AF = mybir.ActivationFunctionType
ALU = mybir.AluOpType

# t_vec scratch layout (free-dim float offsets)
V_S = 0       # BN scale per feature (8)
V_T = 8       # BN shift per feature (8)
V_M = 16      # mean
V_E2 = 24
V_VU = 32     # var, then var+eps
V_SQ = 40     # sqrt(var+eps)
V_R = 48      # rsqrt
V_TMP = 56
V_TMP2 = 64


def build(nc):
    xT = nc.dram_tensor("xT", [KT1, NB, 128, 512], f32r, kind="ExternalInput")
    w1t = nc.dram_tensor("w1t", [D_IN_PAD, H], bf16, kind="ExternalInput")
    w2t = nc.dram_tensor("w2t", [H, H], bf16, kind="ExternalInput")
    w3t = nc.dram_tensor("w3t", [H, H], bf16, kind="ExternalInput")
    w4t = nc.dram_tensor("w4t", [H, D_OUT], bf16, kind="ExternalInput")
    gbp = nc.dram_tensor("gbp", [128, 6, 8], f32, kind="ExternalInput")
    outT = nc.dram_tensor("outT", [D_OUT, BPC], f32, kind="ExternalOutput")

    rg = [list(range(N_CORES))]

    with tile.TileContext(nc) as tc:
        with (
            tc.tile_pool(name="hp", bufs=2) as hpool,
            tc.tile_pool(name="wp", bufs=2) as wpool,
            tc.tile_pool(name="w4p", bufs=1) as w4pool,
            tc.tile_pool(name="stg", bufs=6) as stgpool,
            tc.tile_pool(name="outp", bufs=2) as outpool,
            tc.tile_pool(name="msc", bufs=1) as mpool,
            tc.tile_pool(name="ps", bufs=8, space="PSUM") as pspool,
            tc.tile_pool(name="dram", bufs=1, space="DRAM") as dpool,
        ):
            t_stats = mpool.tile([128, 192], f32, name="t_stats")
            t_part = mpool.tile([128, 16], f32, name="t_part")
            t_gst = mpool.tile([128, 16], f32, name="t_gst")
            t_vec = mpool.tile([128, 72], f32, name="t_vec")
            t_gb = mpool.tile([128, 48], f32, name="t_gb")

            # --- warmup collective: absorb first-call ncfw/algorithm cost.
            # Pure DRAM->DRAM with unread output: zero coupling with the
            # compute DMA queues or SBUF dependency tracking.
            with nc.named_scope("warmup_ar"):
                win = dpool.tile([128, 2], f32, name="warm_in")
                wout = dpool.tile([128, 2], f32, name="warm_out", addr_space="Shared")
                nc.gpsimd.collective_compute(
                    "AllReduce", ALU.add, replica_groups=rg,
                    ins=[win[:].opt()], outs=[wout[:].opt()],
                )

            def prep_w(wt_dram, Wtile, nkt, tag_suffix, after=None):
                """DMA raw transposed weights into staging (half-tiles for a
                finer DMA/Sign pipeline), binarize (Sign). The first Sign is
                order-pinned after `after` so the scheduler cannot hoist the
                sign burst into the previous layer's boundary ACT window."""
                after_inst = [after]
                for k in range(nkt):
                    for hh in range(2):
                        stg = stgpool.tile(
                            [128, H // 2], bf16, name=f"stg_{tag_suffix}_{k}_{hh}", tag="stg"
                        )
                        nc.sync.dma_start(
                            stg[:],
                            wt_dram[k * 128 : (k + 1) * 128, hh * 512 : (hh + 1) * 512],
                        )
                        si = nc.scalar.activation(
                            Wtile[:, k, hh * 512 : (hh + 1) * 512], stg[:], AF.Sign
                        )
                        if after_inst[0] is not None:
                            add_dep_helper(
                                si.ins, after_inst[0], False,
                                "keep boundary ACT ops ahead of weight signs",
                            )
                            after_inst[0] = None

            # --- input loads, in first-consumer order: the layer-1 j0 column
            # needs W1s[k] and xT[b=0, k] for every k first.
            xT_t = hpool.tile([128, KT1, NB, 512], f32r, name="xT_t", tag="h")
            W1s = wpool.tile([128, KT1, H], f32r, name="W1s", tag="w")
            with nc.named_scope("w1prep"):
                for k in range(KT1):
                    for hh in range(2):
                        stg = stgpool.tile(
                            [128, H // 2], bf16, name=f"stg_w1_{k}_{hh}", tag="stg"
                        )
                        nc.sync.dma_start(
                            stg[:],
                            w1t[k * 128 : (k + 1) * 128, hh * 512 : (hh + 1) * 512],
                        )
                        nc.scalar.activation(
                            W1s[:, k, hh * 512 : (hh + 1) * 512], stg[:], AF.Sign
                        )
                    nc.sync.dma_start(xT_t[:, k, 0], xT[k, 0])
            with nc.named_scope("xload"):
                for b in range(1, NB):
                    for k in range(KT1):
                        nc.sync.dma_start(xT_t[:, k, b], xT[k, b])
            nc.sync.dma_start(t_gb[:], gbp[:].rearrange("p a b -> p (a b)"))

            W4s = w4pool.tile([128, NJ, D_OUT], f32r, name="W4s")
            with nc.named_scope("w4prep"):
                stg4 = stgpool.tile([128, NJ, D_OUT], bf16, name="stg_w4", tag="stg")
                nc.sync.dma_start(
                    stg4[:], w4t[:].rearrange("(kt p) c -> p kt c", p=128)
                )
                nc.scalar.activation(
                    W4s[:].rearrange("p a b -> p (a b)"),
                    stg4[:].rearrange("p a b -> p (a b)"),
                    AF.Sign,
                )

            h1 = hpool.tile([128, NJ, NB, 512], f32r, name="h1", tag="h")
            h2 = hpool.tile([128, NJ, NB, 512], f32r, name="h2", tag="h")
            h3 = hpool.tile([128, NJ, NB, 512], f32r, name="h3", tag="h")

            def collective_group(li, j_lo, j_hi, gtag):
                """All-reduce partial stats for feature tiles [j_lo, j_hi)."""
                n = (j_hi - j_lo) * 2
                with nc.named_scope(f"L{li}_ar{gtag}"):
                    cin = dpool.tile([128, n], f32, name=f"cin{li}{gtag}")
                    cout = dpool.tile(
                        [128, n], f32, name=f"cout{li}{gtag}", addr_space="Shared"
                    )
                    nc.gpsimd.dma_start(
                        cin[:], t_part[:, j_lo * 2 : j_hi * 2]
                    )
                    nc.gpsimd.collective_compute(
                        "AllReduce", ALU.add, replica_groups=rg,
                        ins=[cin[:].opt()], outs=[cout[:].opt()],
                    )
                    nc.gpsimd.dma_start(t_gst[:, j_lo * 2 : j_hi * 2], cout[:])

            def st_group(li, j_lo, j_hi, gtag):
                """s = g*rsqrt(v+eps), t = b - m*s for feature tiles [j_lo, j_hi)."""
                with nc.named_scope(f"L{li}_st{gtag}"):
                    gview = t_gst[:, j_lo * 2 : j_hi * 2].rearrange(
                        "p (j c) -> p j c", c=2
                    )
                    mm = t_vec[:, V_M + j_lo : V_M + j_hi]
                    e2 = t_vec[:, V_E2 + j_lo : V_E2 + j_hi]
                    vu = t_vec[:, V_VU + j_lo : V_VU + j_hi]
                    sq = t_vec[:, V_SQ + j_lo : V_SQ + j_hi]
                    rr = t_vec[:, V_R + j_lo : V_R + j_hi]
                    tp2 = t_vec[:, V_TMP2 + j_lo : V_TMP2 + j_hi]
                    sv = t_vec[:, V_S + j_lo : V_S + j_hi]
                    tv = t_vec[:, V_T + j_lo : V_T + j_hi]
                    g_sl = t_gb[:, (li - 1) * 16 + j_lo : (li - 1) * 16 + j_hi]
                    b_sl = t_gb[:, (li - 1) * 16 + 8 + j_lo : (li - 1) * 16 + 8 + j_hi]
                    nc.vector.tensor_scalar(mm, gview[:, :, 0], 1.0 / N_CORES, None, op0=ALU.mult)
                    nc.vector.tensor_scalar(e2, gview[:, :, 1], 1.0 / N_CORES, None, op0=ALU.mult)
                    nc.vector.tensor_tensor(tp2, mm, mm, op=ALU.mult)
                    nc.vector.tensor_tensor(vu, e2, tp2, op=ALU.subtract)
                    nc.vector.tensor_scalar(vu, vu, BN_EPS, None, op0=ALU.add)
                    nc.scalar.activation(sq, vu, AF.Sqrt)
                    nc.vector.reciprocal(rr, sq)
                    nc.vector.tensor_tensor(sv, g_sl, rr, op=ALU.mult)
                    nc.vector.tensor_tensor(tp2, mm, sv, op=ALU.mult)
                    nc.vector.tensor_tensor(tv, b_sl, tp2, op=ALU.subtract)

            def apply_group(li, out_h, j_lo, j_hi):
                last = None
                with nc.named_scope(f"L{li}_apply{j_lo}"):
                    for j in range(j_lo, j_hi):
                        for b in range(NB):
                            last = nc.scalar.activation(
                                out_h[:, j, b],
                                out_h[:, j, b].bitcast(f32),
                                AF.Relu,
                                bias=t_vec[:, V_T + j : V_T + j + 1],
                                scale=t_vec[:, V_S + j : V_S + j + 1],
                            )
                return last

            def layer(li, Wcur, nkt, rhs, out_h, prep_next, split_b0=False):
                """One hidden layer: matmuls + stats + allreduce + BN/ReLU apply."""
                if split_b0:
                    # b0-only warm pass: dense PE work on the first-arriving
                    # rhs chunk while the rest of the input streams in
                    with nc.named_scope(f"L{li}_mm0"):
                        for j in range(NJ):
                            acc = pspool.tile(
                                [128, 512], f32, name=f"ps_l{li}p0_j{j}", tag="ps"
                            )
                            for k in range(nkt):
                                nc.tensor.matmul(
                                    acc[:],
                                    Wcur[:, k, j * 128 : (j + 1) * 128],
                                    rhs[:, k, 0],
                                    start=(k == 0),
                                    stop=(k == nkt - 1),
                                )
                            nc.vector.tensor_copy(out_h[:, j, 0], acc[:])
                b_lo = 1 if split_b0 else 0
                with nc.named_scope(f"L{li}_mm"):
                    for j in range(NJ):
                        accs = [
                            pspool.tile(
                                [128, 512], f32, name=f"ps_l{li}_j{j}_b{b}", tag="ps"
                            )
                            for b in range(b_lo, NB)
                        ]
                        for b in range(b_lo, NB):
                            for k in range(nkt):
                                nc.tensor.matmul(
                                    accs[b - b_lo][:],
                                    Wcur[:, k, j * 128 : (j + 1) * 128],
                                    rhs[:, k, b],
                                    start=(k == 0),
                                    stop=(k == nkt - 1),
                                )
                        for b in range(b_lo, NB):
                            nc.vector.tensor_copy(out_h[:, j, b], accs[b - b_lo][:])
                        for b in range(NB):
                            so = j * 24 + b * 6
                            nc.vector.bn_stats(
                                t_stats[:, so : so + 6], out_h[:, j, b].bitcast(f32)
                            )
                        # per-j partial: bn_aggr -> (mean, var); then E2 = var + mean^2
                        po = j * 2
                        nc.vector.bn_aggr(
                            t_part[:, po : po + 2],
                            t_stats[:, j * 24 : j * 24 + 24],
                        )
                        nc.vector.tensor_tensor(
                            t_vec[:, V_TMP + j : V_TMP + j + 1],
                            t_part[:, po : po + 1],
                            t_part[:, po : po + 1],
                            op=ALU.mult,
                        )
                        nc.vector.tensor_tensor(
                            t_part[:, po + 1 : po + 2],
                            t_vec[:, V_TMP + j : V_TMP + j + 1],
                            t_part[:, po + 1 : po + 2],
                            op=ALU.add,
                        )
                        if j == 1 and prep_next is not None:
                            prep_next(prev_apply[0])
                        if j == NJ - 3:
                            # early group: all-reduce features 0..(NJ-2);
                            # finishes during the j6/j7 tail
                            collective_group(li, 0, NJ - 2, "a")
                        if j == NJ - 2:
                            # emitted before j7's drains so the DVE stream
                            # does the s,t math as soon as the data is back
                            st_group(li, 0, NJ - 2, "a")
                            apply_group(li, out_h, 0, NJ - 2)
                # late group: the last two feature tiles
                collective_group(li, NJ - 2, NJ, "b")
                st_group(li, NJ - 2, NJ, "b")
                return apply_group(li, out_h, NJ - 2, NJ)

            W2s = wpool.tile([128, NJ, H], f32r, name="W2s", tag="w")
            W3s = wpool.tile([128, NJ, H], f32r, name="W3s", tag="w")

            prev_apply = [None]
            a1 = layer(1, W1s, KT1, xT_t, h1,
                       lambda after: prep_w(w2t, W2s, NJ, "w2", after),
                       split_b0=True)
            prev_apply[0] = a1.ins if a1 is not None else None
            a2 = layer(2, W2s, NJ, h1, h2,
                       lambda after: prep_w(w3t, W3s, NJ, "w3", after))
            prev_apply[0] = a2.ins if a2 is not None else None
            layer(3, W3s, NJ, h2, h3, None)

            # ---- head: 10-wide binarized linear + sigmoid ------------------
            with nc.named_scope("L4"):
                for b in range(NB):
                    acc = pspool.tile([D_OUT, 512], f32, name=f"ps_l4_b{b}", tag="ps")
                    for k in range(NJ):
                        nc.tensor.matmul(
                            acc[:],
                            W4s[:, k],
                            h3[:, k, b],
                            start=(k == 0),
                            stop=(k == NJ - 1),
                        )
                    osb = outpool.tile([D_OUT, 512], f32, name=f"osb{b}", tag="osb")
                    nc.scalar.activation(osb[:], acc[:], AF.Sigmoid)
                    nc.sync.dma_start(outT[:, b * 512 : (b + 1) * 512], osb[:])

    nc.compile()
    return nc


_NC = None
_LAST_RESULTS = None


def _get_nc():
    global _NC
    if _NC is None:
        nc = bacc.Bacc(
            "TRN2", target_bir_lowering=False, debug=False, num_devices=N_CORES
        )
        build(nc)
        _NC = nc
    return _NC


def kernel(**inputs):
    x = np.ascontiguousarray(inputs["x"], dtype=np.float32)
    w1 = np.asarray(inputs["w1"], dtype=np.float32)
    w2 = np.asarray(inputs["w2"], dtype=np.float32)
    w3 = np.asarray(inputs["w3"], dtype=np.float32)
    w4 = np.asarray(inputs["w4"], dtype=np.float32)
    gb = np.stack(
        [
            np.asarray(inputs[n], dtype=np.float32)
            for n in ("g1", "b1", "g2", "b2", "g3", "b3")
        ]
    )  # [6, 1024]

    import ml_dtypes

    bf = ml_dtypes.bfloat16
    w1t = np.zeros((D_IN_PAD, H), bf)
    w1t[:D_IN] = w1.T.astype(bf)
    w2t = np.ascontiguousarray(w2.T.astype(bf))
    w3t = np.ascontiguousarray(w3.T.astype(bf))
    w4t = np.ascontiguousarray(w4.T.astype(bf))
    gbp = np.ascontiguousarray(gb.reshape(6, 8, 128).transpose(2, 0, 1))  # [128,6,8]

    nc = _get_nc()
    in_maps = []
    for c in range(N_CORES):
        xs = np.zeros((D_IN_PAD, BPC), np.float32)
        xs[:D_IN] = x[c * BPC : (c + 1) * BPC].T
        xs = np.ascontiguousarray(
            xs.reshape(KT1, 128, NB, 512).transpose(0, 2, 1, 3)
        )
        in_maps.append(
            {"xT": xs, "w1t": w1t, "w2t": w2t, "w3t": w3t, "w4t": w4t, "gbp": gbp}
        )

    res = run_bass_kernel_spmd(nc, in_maps, core_ids=list(range(N_CORES)))
    global _LAST_RESULTS
    _LAST_RESULTS = res
    out = np.empty((B_TOT, D_OUT), dtype=np.float32)
    for c in range(N_CORES):
        out[c * BPC : (c + 1) * BPC] = res.results[c]["outT"].T
    return out
